# revision 4
# baseline (speedup 1.0000x reference)
"""Trainium2 Bass kernel for Tacotron2-style location-sensitive attention.

Reference computation (per batch b):
  conv   = conv1d(attention_weight_cat, conv_w, SAME) + conv_b  -> batchnorm(eval)
  loc    = einsum('bft,fa->bta', conv, Wloc)
  q      = query @ Wq
  k      = einsum('bte,ea->bta', key, Wk)
  e      = tanh(q + k + loc) @ v ; mask ; softmax over t
  ctx    = einsum('bt,bte->be', softmax, key)

Strategy: data-parallel over batch across 8 NeuronCores (8 batch rows each).
Inside a core everything is fp32 data using fp32r matmuls (full-rate PE).
The conv+BN+Wloc chain is folded host-side into a single [62,128] matrix so
location features become one im2col matmul, accumulated into the same PSUM
bank as the key projection; ScalarE applies tanh with the (q + loc-bias)
per-partition bias straight out of PSUM.
"""

import os
import sys
import dataclasses
from contextlib import ExitStack

import numpy as np

for _p in ("/opt/trn_rl_repo", "/root/.axon_site/_ro/trn_rl_repo"):
    if os.path.isdir(_p) and _p not in sys.path:
        sys.path.insert(0, _p)

B, T, RNN_D, EMB_D, ATT_D, NF, KW = 64, 1000, 1024, 512, 128, 32, 31
N_CORES = 8
BL = B // N_CORES  # batch rows per core
BN_EPS = 1e-5
KC = 2 * KW  # im2col contraction size (62)

# t-blocking: 8 blocks of 128 (last=104); 2 column groups of the staging width
TBLOCKS = [(tb, 128 if tb < 7 else T - 7 * 128) for tb in range(8)]
TGROUPS = [(0, range(0, 4), 0, 512), (1, range(4, 8), 512, T - 512)]

_CACHE: dict = {}


def _build_nc():
    import concourse.bass as bass
    import concourse.mybir as mybir
    import concourse.tile as tile
    from concourse import bacc
    from concourse.masks import make_identity

    f32 = mybir.dt.float32
    f32r = mybir.dt.float32r
    AF = mybir.ActivationFunctionType

    def rep(ap, dims, doff=0):
        return dataclasses.replace(ap, offset=ap.offset + doff, ap=dims)

    nc = bacc.Bacc(
        "TRN2", target_bir_lowering=False, debug=False, num_devices=N_CORES
    )

    key_d = nc.dram_tensor("key", [BL, T, EMB_D], f32, kind="ExternalInput").ap()
    query_d = nc.dram_tensor("query", [BL, RNN_D], f32, kind="ExternalInput").ap()
    aw_d = nc.dram_tensor("aw", [BL, 2, T], f32, kind="ExternalInput").ap()
    maskb_d = nc.dram_tensor("maskb", [BL, T], f32, kind="ExternalInput").ap()
    wq_d = nc.dram_tensor("wq", [RNN_D, ATT_D], f32, kind="ExternalInput").ap()
    wk_d = nc.dram_tensor("wk", [EMB_D, ATT_D], f32, kind="ExternalInput").ap()
    wcomb_d = nc.dram_tensor("wcomb", [KC, ATT_D], f32, kind="ExternalInput").ap()
    locbias_d = nc.dram_tensor("locbias", [ATT_D, 1], f32, kind="ExternalInput").ap()
    vmat_d = nc.dram_tensor("vmat", [ATT_D, BL * BL], f32, kind="ExternalInput").ap()
    ctx_o = nc.dram_tensor("ctx", [BL, EMB_D], f32, kind="ExternalOutput").ap()
    w_o = nc.dram_tensor("weights", [BL, T], f32, kind="ExternalOutput").ap()

    with ExitStack() as ctx:
        tc = ctx.enter_context(tile.TileContext(nc))
        const = ctx.enter_context(tc.tile_pool(name="const", bufs=1))
        keyp = ctx.enter_context(tc.tile_pool(name="keyp", bufs=1))
        stage = ctx.enter_context(tc.tile_pool(name="stage", bufs=2))
        tanhp = ctx.enter_context(tc.tile_pool(name="tanhp", bufs=2))
        smp = ctx.enter_context(tc.tile_pool(name="smp", bufs=1))
        i2cp = ctx.enter_context(tc.tile_pool(name="i2cp", bufs=2))
        outp = ctx.enter_context(tc.tile_pool(name="outp", bufs=1))
        psT = ctx.enter_context(tc.tile_pool(name="psT", bufs=2, space="PSUM"))
        psQKL = ctx.enter_context(tc.tile_pool(name="psQKL", bufs=2, space="PSUM"))
        psE = ctx.enter_context(tc.tile_pool(name="psE", bufs=1, space="PSUM"))
        psM = ctx.enter_context(tc.tile_pool(name="psM", bufs=1, space="PSUM"))

        # ---- constants / parameters -------------------------------------
        ident_f = const.tile([128, 128], f32)
        make_identity(nc, ident_f[:])
        ident = const.tile([128, 128], f32r)
        nc.sync.dma_start(out=ident[:], in_=ident_f[:].bitcast(f32r))

        wk_t = []
        for c in range(4):
            t_ = const.tile([128, ATT_D], f32r, tag=f"wk{c}")
            nc.sync.dma_start(
                out=t_[:], in_=wk_d[128 * c : 128 * (c + 1), :].bitcast(f32r)
            )
            wk_t.append(t_)
        wq_t = []
        for c in range(8):
            t_ = const.tile([128, ATT_D], f32r, tag=f"wq{c}")
            nc.sync.dma_start(
                out=t_[:], in_=wq_d[128 * c : 128 * (c + 1), :].bitcast(f32r)
            )
            wq_t.append(t_)
        wcomb_t = const.tile([KC, ATT_D], f32r)
        nc.sync.dma_start(out=wcomb_t[:], in_=wcomb_d.bitcast(f32r))
        vmat_t = const.tile([ATT_D, BL * BL], f32r)
        nc.sync.dma_start(out=vmat_t[:], in_=vmat_d.bitcast(f32r))
        locbias_t = const.tile([ATT_D, 1], f32)
        nc.sync.dma_start(out=locbias_t[:], in_=locbias_d)
        maskb_t = const.tile([BL, T], f32)
        nc.sync.dma_start(out=maskb_t[:], in_=maskb_d)

        # ---- query projection -> per-partition bias [a, b] --------------
        query_t = const.tile([BL, RNN_D], f32r)
        nc.sync.dma_start(out=query_t[:], in_=query_d.bitcast(f32r))
        q_ps = psM.tile([ATT_D, BL], f32, tag="ctx", name="q_ps")
        for c in range(8):
            qT_ps = psM.tile([128, BL], f32r, tag="small")
            nc.tensor.transpose(
                out=qT_ps[:],
                in_=query_t[0:BL, 128 * c : 128 * (c + 1)],
                identity=ident[0:BL, 0:BL],
            )
            qT_sb = stage.tile([128, BL], f32r, tag="qT")
            nc.vector.tensor_copy(out=qT_sb[:], in_=qT_ps[:])
            nc.tensor.matmul(
                q_ps[:], wq_t[c][:], qT_sb[:], start=(c == 0), stop=(c == 7)
            )
        qbias = const.tile([ATT_D, BL], f32)
        nc.vector.tensor_scalar_add(out=qbias[:], in0=q_ps[:], scalar1=locbias_t[:])

        # ---- key shard: natural layout, resident in SBUF ----------------
        key_t = []
        for b in range(BL):
            kt = keyp.tile([128, 8 * EMB_D], f32r, tag=f"key{b}")
            nc.sync.dma_start(
                out=kt[:, 0 : 7 * EMB_D],
                in_=rep(
                    key_d[b],
                    [[EMB_D, 128], [128 * EMB_D, 7], [1, EMB_D]],
                ).bitcast(f32r),
            )
            nc.sync.dma_start(
                out=kt[0:104, 7 * EMB_D : 8 * EMB_D],
                in_=rep(
                    key_d[b], [[EMB_D, 104], [1, EMB_D]], doff=896 * EMB_D
                ).bitcast(f32r),
            )
            key_t.append(kt)

        # ---- conv im2col staging ----------------------------------------
        aw_pad = const.tile([2 * BL, T + KW - 1], f32)
        nc.vector.memset(aw_pad[:], 0.0)
        nc.sync.dma_start(
            out=aw_pad[:, KW // 2 : KW // 2 + T],
            in_=rep(aw_d, [[T, 2 * BL], [1, T]]),
        )
        i2c_t = []
        for b in range(BL):
            it = i2cp.tile([KC, T], f32r, tag="i2c")
            src = aw_pad[2 * b : 2 * b + 2, 0:T]
            nc.sync.dma_start(
                out=it[:],
                in_=rep(src, [src.ap[0], [1, KW], [1, T]]).bitcast(f32r),
            )
            i2c_t.append(it)

        # ---- main per-(b, tgroup) pipeline ------------------------------
        psE_t = {
            tg: psE.tile([BL, tgw], f32, tag=f"e{tg}", name=f"psE{tg}")
            for tg, _, _, tgw in TGROUPS
        }
        for b in range(BL):
            for tg, tbs, tg0, tgw in TGROUPS:
                kT_sb = []
                for c in range(4):
                    pT = psT.tile([128, tgw], f32r, tag="pT")
                    for j, tb in enumerate(tbs):
                        tw = TBLOCKS[tb][1]
                        nc.tensor.transpose(
                            out=pT[:, 128 * j : 128 * j + tw],
                            in_=key_t[b][
                                0:tw, EMB_D * tb + 128 * c : EMB_D * tb + 128 * (c + 1)
                            ],
                            identity=ident[0:tw, 0:tw],
                        )
                    kt_sb = stage.tile([128, tgw], f32r, tag=f"kT{c}")
                    nc.any.tensor_copy(out=kt_sb[:], in_=pT[:])
                    kT_sb.append(kt_sb)
                pqkl = psQKL.tile([ATT_D, tgw], f32, tag="qkl")
                for c in range(4):
                    nc.tensor.matmul(
                        pqkl[:], wk_t[c][:], kT_sb[c][:], start=(c == 0), stop=False
                    )
                nc.tensor.matmul(
                    pqkl[:],
                    wcomb_t[:],
                    i2c_t[b][:, tg0 : tg0 + tgw],
                    start=False,
                    stop=True,
                )
                th = tanhp.tile([ATT_D, tgw], f32r, tag="th")
                nc.scalar.activation(
                    out=th[:],
                    in_=pqkl[:],
                    func=AF.Tanh,
                    bias=qbias[:, b : b + 1],
                    scale=1.0,
                )
                nc.tensor.matmul(
                    psE_t[tg][:],
                    vmat_t[:, BL * b : BL * (b + 1)],
                    th[:],
                    start=(b == 0),
                    stop=(b == BL - 1),
                    skip_group_check=True,
                )

        # ---- mask + softmax ---------------------------------------------
        e_sb = smp.tile([BL, T], f32, tag="esb")
        for tg, _, tg0, tgw in TGROUPS:
            nc.vector.tensor_add(
                out=e_sb[:, tg0 : tg0 + tgw],
                in0=psE_t[tg][:],
                in1=maskb_t[:, tg0 : tg0 + tgw],
            )
        negm = smp.tile([BL, 1], f32, tag="negm")
        nc.vector.tensor_reduce(
            out=negm[:],
            in_=e_sb[:],
            axis=mybir.AxisListType.X,
            op=mybir.AluOpType.max,
            negate=True,
        )
        expr = smp.tile([BL, T], f32r, tag="expr")
        sumexp = smp.tile([BL, 1], f32, tag="sumexp")
        nc.scalar.activation(
            out=expr[:],
            in_=e_sb[:],
            func=AF.Exp,
            bias=negm[:],
            scale=1.0,
            accum_out=sumexp[:],
        )
        r_sb = smp.tile([BL, 1], f32, tag="rsb")
        nc.vector.reciprocal(out=r_sb[:], in_=sumexp[:])
        w32 = smp.tile([BL, T], f32, tag="esb")
        nc.vector.tensor_scalar_mul(out=w32[:], in0=expr[:], scalar1=r_sb[:])
        nc.sync.dma_start(out=w_o, in_=w32[:])

        # ---- context: ctx = (sum_t exp * key) / sumexp -------------------
        embz = const.tile([128, (BL + 1) * BL], f32)
        nc.vector.memset(embz[:], 0.0)
        emb_ab = []
        for i in range(2):
            e_ = const.tile([128, (BL + 1) * BL], f32r, tag=f"emb{i}")
            nc.sync.dma_start(out=e_[:], in_=embz[:].bitcast(f32r))
            emb_ab.append(e_)

        ps_ctx = psM.tile([BL, EMB_D], f32, tag="ctx")
        for tb, tw in TBLOCKS:
            wT_ps = psM.tile([128, BL], f32r, tag="small")
            nc.tensor.transpose(
                out=wT_ps[0:tw, :],
                in_=expr[0:BL, 128 * tb : 128 * tb + tw],
                identity=ident[0:BL, 0:BL],
            )
            emb = emb_ab[tb % 2]
            demb = emb[:]
            nc.vector.tensor_copy(
                out=rep(demb, [[demb.ap[0][0], tw], [BL + 1, BL]]),
                in_=wT_ps[0:tw, :],
            )
            for b in range(BL):
                nc.tensor.matmul(
                    ps_ctx[:],
                    emb[0:tw, BL * b : BL * (b + 1)],
                    key_t[b][0:tw, EMB_D * tb : EMB_D * (tb + 1)],
                    start=(tb == 0 and b == 0),
                    stop=(tb == 7 and b == BL - 1),
                    skip_group_check=True,
                )
        ctx_sb = outp.tile([BL, EMB_D], f32, tag="ctx")
        nc.vector.tensor_scalar_mul(out=ctx_sb[:], in0=ps_ctx[:], scalar1=r_sb[:])
        nc.sync.dma_start(out=ctx_o, in_=ctx_sb[:])

    nc.compile()
    return nc


def _get_nc():
    if "nc" not in _CACHE:
        _CACHE["nc"] = _build_nc()
    return _CACHE["nc"]


def _host_prep(query, key, attention_weight_cat, mask, Wq, Wk, conv_w, conv_b,
               bn_gamma, bn_beta, bn_mean, bn_var, Wloc, v):
    """Fold conv+BN+Wloc into one matrix; build per-core input maps."""
    f8 = np.float64
    scale = (bn_gamma.astype(f8) / np.sqrt(bn_var.astype(f8) + BN_EPS))
    # conv_w: [NF, 2, KW] -> CW[k=(c,dk), f] scaled by BN
    cw = (conv_w.astype(f8) * scale[:, None, None]).transpose(1, 2, 0).reshape(KC, NF)
    cb = (conv_b.astype(f8) - bn_mean.astype(f8)) * scale + bn_beta.astype(f8)
    wcomb = np.ascontiguousarray((cw @ Wloc.astype(f8)).astype(np.float32))
    locbias = (cb @ Wloc.astype(f8)).astype(np.float32).reshape(ATT_D, 1)
    vmat = np.zeros((ATT_D, BL * BL), np.float32)
    for b in range(BL):
        vmat[:, BL * b + b] = v
    maskb = np.where(mask, np.float32(-1e30), np.float32(0.0))

    common = dict(
        wq=np.ascontiguousarray(Wq, np.float32),
        wk=np.ascontiguousarray(Wk, np.float32),
        wcomb=wcomb,
        locbias=locbias,
        vmat=vmat,
    )
    in_maps = []
    for c in range(N_CORES):
        s = slice(BL * c, BL * (c + 1))
        in_maps.append(
            dict(
                key=np.ascontiguousarray(key[s], np.float32),
                query=np.ascontiguousarray(query[s], np.float32),
                aw=np.ascontiguousarray(attention_weight_cat[s], np.float32),
                maskb=np.ascontiguousarray(maskb[s], np.float32),
                **common,
            )
        )
    return in_maps


def kernel(query, key, attention_weight_cat, mask, Wq, Wk, conv_w, conv_b,
           bn_gamma, bn_beta, bn_mean, bn_var, Wloc, v):
    from concourse.bass_utils import run_bass_kernel_spmd

    nc = _get_nc()
    in_maps = _host_prep(query, key, attention_weight_cat, mask, Wq, Wk, conv_w,
                         conv_b, bn_gamma, bn_beta, bn_mean, bn_var, Wloc, v)
    res = run_bass_kernel_spmd(nc, in_maps, core_ids=list(range(N_CORES)))
    ctx = np.concatenate([res.results[c]["ctx"] for c in range(N_CORES)], axis=0)
    wts = np.concatenate([res.results[c]["weights"] for c in range(N_CORES)], axis=0)
    return ctx, wts
